# revision 48
# baseline (speedup 1.0000x reference)
"""Trainium2 Bass kernel for nn_Decoder (single-query MHA + pointer head).

Contract: kernel(**inputs) takes the FULL unsharded numpy inputs (as produced
by the problem's setup_inputs) and returns the full output (vertexes, probs),
matching the reference up to fp32 rounding.

Strategy (pure data parallelism over batch, 8 NeuronCores, 32 batch each):
  - Host does LAYOUT ONLY: batch-slice, concat h_c, transposes of V/K_lg,
    zero padding, mask replication. All math runs on device in fp32.
  - Per core, partition layout (b_local, head) on 128 partitions x 2 blocks:
    scores (K dot q, grouped reduce) and the attention-weighted V sum run on
    Vector/GpSimd with n in the free dimension (row softmax is native).
  - Q / Wo projections run on the Tensor engine; pointer logits use M=1
    matmuls with the u2 column stationary and K_lg.T streaming as the moving
    operand, drained via scalar copies + small scatter DMAs.
  - argmax via DVE max/max_index (first-index tie-break == jnp.argmax).
"""

import numpy as np

B, N, D, H, HD = 256, 1024, 128, 8, 16
NCORES = 8
BPC = B // NCORES          # 32 batches per core
BLK_B = 16                 # batches per partition-block (16 b x 8 h = 128)
NBLK = BPC // BLK_B        # 2
KPAD = 512                 # 386 -> 512 (4 chunks of 128) for Q projection
NEG = -1.0e15
RSQ_D = float(1.0 / np.sqrt(128.0))

_PROG_CACHE = {}


def _build_program():
    """Build the (SPMD-identical) Bass program once."""
    import concourse.bass as bass
    import concourse.bacc as bacc
    import concourse.mybir as mybir
    from concourse.tile import TileContext

    f32 = mybir.dt.float32
    i32 = mybir.dt.int32
    u32 = mybir.dt.uint32
    Alu = mybir.AluOpType
    Act = mybir.ActivationFunctionType
    Ax = mybir.AxisListType

    # Bacc (not plain Bass): its compile() pass legalizes instruction-attached
    # semaphore waits (move_matmul_waits_to_ldweights, event semaphores) that
    # walrus codegen otherwise rejects ("Too many sync wait commands").
    nc = bacc.Bacc(None, target_bir_lowering=False)

    # ---- DRAM parameters (per-core) ----
    hcT = nc.declare_dram_parameter("hcT", [KPAD, BPC], f32, isOutput=False)
    wqT = nc.declare_dram_parameter("wqT", [KPAD, D], f32, isOutput=False)
    bq = nc.declare_dram_parameter("bq", [D, 1], f32, isOutput=False)
    woT = nc.declare_dram_parameter("woT", [D, D], f32, isOutput=False)
    bo = nc.declare_dram_parameter("bo", [D, 1], f32, isOutput=False)
    ident = nc.declare_dram_parameter("ident", [128, 128], f32, isOutput=False)
    Kn = nc.declare_dram_parameter("Kn", [BPC * H, N * HD], f32, isOutput=False)
    Vt = nc.declare_dram_parameter("Vt", [BPC * H, HD * N], f32, isOutput=False)
    KlgT = nc.declare_dram_parameter("KlgT", [BPC, D, N], f32, isOutput=False)
    mrep = nc.declare_dram_parameter("mrep", [BPC * H, N], i32, isOutput=False)
    m32 = nc.declare_dram_parameter("m32", [BPC, N], i32, isOutput=False)
    vert_out = nc.declare_dram_parameter("verts", [BPC, 1], i32, isOutput=True)
    probs_out = nc.declare_dram_parameter("probs", [BPC, 1], f32, isOutput=True)

    NC4 = 4096             # K/V free elems per chunk tile (256 n x 16 d)
    NCH = N // 256         # 4 chunks

    with TileContext(nc) as tc:
        import contextlib

        with contextlib.ExitStack() as ctx:
            const_p = ctx.enter_context(tc.tile_pool(name="const", bufs=1))
            small_p = ctx.enter_context(tc.tile_pool(name="small", bufs=1))
            blk_p = ctx.enter_context(tc.tile_pool(name="blk", bufs=2))
            kstream = ctx.enter_context(tc.tile_pool(name="kstream", bufs=2))
            vstream = ctx.enter_context(tc.tile_pool(name="vstream", bufs=2))
            prod_p = ctx.enter_context(tc.tile_pool(name="prod", bufs=2))
            klg_p = ctx.enter_context(tc.tile_pool(name="klg", bufs=3))
            psum_p = ctx.enter_context(
                tc.tile_pool(name="psum", bufs=1, space=bass.MemorySpace.PSUM)
            )
            psum_tr = ctx.enter_context(
                tc.tile_pool(name="psumtr", bufs=2, space=bass.MemorySpace.PSUM)
            )
            psum_lg = ctx.enter_context(
                tc.tile_pool(name="psumlg", bufs=4, space=bass.MemorySpace.PSUM)
            )
            dram_p = ctx.enter_context(
                tc.tile_pool(name="dram", bufs=1, space=bass.MemorySpace.DRAM)
            )

            # ---------- Phase Q: Q = 0.25*(h_c @ Wq.T + bq), per (b,h) ----------
            ident_t = const_p.tile([128, 128], f32)
            nc.sync.dma_start(ident_t[:], ident[:])

            bq_t = const_p.tile([D, 1], f32)
            nc.sync.dma_start(bq_t[:], bq[:])
            bq25 = const_p.tile([D, 1], f32)
            nc.scalar.mul(bq25[:], bq_t[:], 0.25)

            qt_ps = psum_p.tile([D, BPC], f32)  # Q.T accumulate over k-chunks
            for kc in range(KPAD // 128):
                wq_t = blk_p.tile([128, D], f32, name="wq_t")
                nc.sync.dma_start(wq_t[:], wqT[kc * 128:(kc + 1) * 128, :])
                hc_t = blk_p.tile([128, BPC], f32, name="hc_t")
                nc.sync.dma_start(hc_t[:], hcT[kc * 128:(kc + 1) * 128, :])
                nc.tensor.matmul(
                    qt_ps[:], wq_t[:], hc_t[:],
                    start=(kc == 0), stop=(kc == KPAD // 128 - 1),
                )
            qt_s = small_p.tile([D, BPC], f32)  # 0.25*(Q.T + bq), [(h d), b]
            nc.scalar.activation(qt_s[:], qt_ps[:], Act.Identity,
                                 bias=bq25[:, 0:1], scale=0.25)

            # transpose -> Q [b, (h d)] and roundtrip via DRAM to [(b h), d]
            q_tr_ps = psum_p.tile([BPC, D], f32)
            nc.tensor.transpose(q_tr_ps[:], qt_s[:], ident_t[:])
            q_sb = small_p.tile([BPC, D], f32)
            nc.scalar.copy(q_sb[:], q_tr_ps[:])
            q_dram = dram_p.tile([BPC, D], f32)
            nc.sync.dma_start(q_dram[:], q_sb[:])

            u_dram = dram_p.tile([BPC, D], f32)
            u2s = small_p.tile([D, BPC], f32)  # (Wo u + bo)/sqrt(D), [(d2), b]

            bo_t = const_p.tile([D, 1], f32)
            nc.sync.dma_start(bo_t[:], bo[:])
            bo_s = const_p.tile([D, 1], f32)
            nc.scalar.mul(bo_s[:], bo_t[:], RSQ_D)

            wo_t = const_p.tile([D, D], f32)
            nc.sync.dma_start(wo_t[:], woT[:])

            # pointer-logits staging: psum rows drained into [b, n]
            logits_sb = small_p.tile([BPC, N], f32)

            for blk in range(NBLK):
                rows = slice(blk * 128, (blk + 1) * 128)

                q_tile = blk_p.tile([128, HD], f32, name="q_tile")
                nc.sync.dma_start(
                    q_tile[:],
                    q_dram[blk * BLK_B:(blk + 1) * BLK_B, :]
                    .rearrange("b (h d) -> (b h) d", h=H),
                )

                # masks for this block
                mrep_t = blk_p.tile([128, N], i32, name="mrep_t")
                nc.sync.dma_start(mrep_t[:], mrep[rows, :])
                m01 = blk_p.tile([128, N], f32, name="m01")
                nc.gpsimd.tensor_copy(m01[:], mrep_t[:])
                mneg = blk_p.tile([128, N], f32, name="mneg")
                nc.gpsimd.tensor_scalar(
                    out=mneg[:], in0=m01[:], scalar1=-1.0, scalar2=-NEG,
                    op0=Alu.add, op1=Alu.mult,
                )

                # ---------- scores: s[(b h), n] = sum_d K * q ----------
                scores_raw = blk_p.tile([128, N], f32, name="scores_raw")
                for c in range(NCH):
                    ktile = kstream.tile([128, NC4], f32, name="ktile")
                    nc.sync.dma_start(
                        ktile[:], Kn[rows, c * NC4:(c + 1) * NC4])
                    kprod = prod_p.tile([128, NC4], f32, name="kprod",
                                        tag="prod")
                    kv = ktile[:].rearrange("p (n d) -> p n d", d=HD)
                    kp = kprod[:].rearrange("p (n d) -> p n d", d=HD)
                    qb = q_tile[:].unsqueeze(1)
                    # split the multiply across Vector (rows 0:64) and
                    # GpSimd (rows 64:128) so both engines work each chunk
                    nc.vector.tensor_tensor(
                        out=kp[0:64], in0=kv[0:64],
                        in1=qb[0:64].broadcast_to([64, 256, HD]),
                        op=Alu.mult,
                    )
                    nc.gpsimd.tensor_tensor(
                        out=kp[64:128], in0=kv[64:128],
                        in1=qb[64:128].broadcast_to([64, 256, HD]),
                        op=Alu.mult,
                    )
                    nc.vector.tensor_reduce(
                        out=scores_raw[:, c * 256:(c + 1) * 256],
                        in_=kprod[:].rearrange("p (n d) -> p n d", d=HD),
                        axis=Ax.X, op=Alu.add,
                    )

                # masked scores (in place): scores_raw += mneg
                nc.vector.tensor_tensor(
                    out=scores_raw[:], in0=scores_raw[:], in1=mneg[:],
                    op=Alu.add)

                negmax = blk_p.tile([128, 1], f32, name="negmax")
                nc.vector.tensor_reduce(
                    out=negmax[:], in_=scores_raw[:], axis=Ax.X, op=Alu.max,
                    negate=True)

                e2 = blk_p.tile([128, N], f32, name="e2")
                nc.scalar.activation(e2[:], scores_raw[:], Act.Exp,
                                     bias=negmax[:, 0:1])
                # zero out masked lanes exactly (in place)
                nc.vector.tensor_tensor(out=e2[:], in0=e2[:], in1=m01[:],
                                        op=Alu.mult)
                s_sum = blk_p.tile([128, 1], f32, name="s_sum")
                nc.vector.tensor_reduce(out=s_sum[:], in_=e2[:], axis=Ax.X,
                                        op=Alu.add)
                rec_s = blk_p.tile([128, 1], f32, name="rec_s")
                nc.vector.reciprocal(rec_s[:], s_sum[:])

                # ---------- u[(b h), d] = (sum_n e2 * V) / S ----------
                part4 = blk_p.tile([128, HD, NCH], f32, name="part4")
                for c in range(NCH):
                    vtile = vstream.tile([128, NC4], f32, name="vtile")
                    nc.scalar.dma_start(
                        vtile[:].rearrange("p (d n) -> p d n", n=256),
                        Vt[rows, :].rearrange("p (d n) -> p d n", n=N)
                        [:, :, c * 256:(c + 1) * 256],
                    )
                    vprod = prod_p.tile([128, NC4], f32, name="vprod",
                                        tag="prod")
                    nc.vector.tensor_tensor(
                        out=vprod[:].rearrange("p (d n) -> p d n", n=256),
                        in0=vtile[:].rearrange("p (d n) -> p d n", n=256),
                        in1=e2[:, c * 256:(c + 1) * 256].unsqueeze(1)
                        .broadcast_to([128, HD, 256]),
                        op=Alu.mult,
                    )
                    nc.vector.tensor_reduce(
                        out=part4[:, :, c],
                        in_=vprod[:].rearrange("p (d n) -> p d n", n=256),
                        axis=Ax.X, op=Alu.add,
                    )
                usum = blk_p.tile([128, HD], f32, name="usum")
                nc.vector.tensor_reduce(out=usum[:], in_=part4[:], axis=Ax.X,
                                        op=Alu.add)
                u_blk = blk_p.tile([128, HD], f32, name="u_blk")
                nc.vector.tensor_tensor(
                    out=u_blk[:], in0=usum[:],
                    in1=rec_s[:, 0:1].broadcast_to([128, HD]), op=Alu.mult)

                nc.sync.dma_start(
                    u_dram[blk * BLK_B:(blk + 1) * BLK_B, :]
                    .rearrange("b (h d) -> (b h) d", h=H),
                    u_blk[:],
                )

                # ---------- u2 for this block: [(d2), b_blk] ----------
                u_plain = blk_p.tile([BLK_B, D], f32, name="u_plain")
                nc.sync.dma_start(
                    u_plain[:], u_dram[blk * BLK_B:(blk + 1) * BLK_B, :])
                uT_ps = psum_tr.tile([D, BLK_B], f32, name="uT_ps", bufs=1)
                nc.tensor.transpose(uT_ps[:], u_plain[:],
                                    ident_t[:BLK_B, :BLK_B])
                uT_sb = blk_p.tile([D, BLK_B], f32, name="uT_sb")
                nc.scalar.copy(uT_sb[:], uT_ps[:])
                u2_ps = psum_tr.tile([D, BLK_B], f32, name="u2_ps", bufs=1)
                nc.tensor.matmul(u2_ps[:], wo_t[:], uT_sb[:])
                nc.scalar.activation(
                    u2s[:, blk * BLK_B:(blk + 1) * BLK_B], u2_ps[:],
                    Act.Identity, bias=bo_s[:, 0:1], scale=RSQ_D)

                # ---------- pointer logits for this block's batches ----------
                # u2 column is the (tiny) stationary operand; K_lg.T streams
                # through the PE as the moving operand at full rate. The psum
                # row (partition 0) is staged to SBUF by the scalar engine,
                # then a small DMA scatters it into logits_sb[b].
                for bl in range(0, BLK_B, 2):
                    b = blk * BLK_B + bl
                    klg_t = klg_p.tile([D, 2 * N], f32, name="klg_t")
                    nc.sync.dma_start(
                        klg_t[:].rearrange("d (two n) -> d two n", two=2),
                        KlgT[b:b + 2, :, :].rearrange("two d n -> d two n"),
                    )
                    for j in range(2):
                        bb = b + j
                        stage = blk_p.tile([1, N], f32, name="lgrow", bufs=4)
                        for c in range(2):
                            lg_ps = psum_lg.tile([1, 512], f32, name="lg_ps")
                            nc.tensor.matmul(
                                lg_ps[:],
                                u2s[:, bb:bb + 1],
                                klg_t[:, j * N + c * 512:j * N + (c + 1) * 512],
                                start=True, stop=True,
                            )
                            nc.scalar.copy(
                                stage[0:1, c * 512:(c + 1) * 512], lg_ps[:])
                        nc.scalar.dma_start(
                            logits_sb[bb:bb + 1, :], stage[0:1, :])

            # ---------- finish pointer head on [b, n] ----------
            nc.scalar.activation(logits_sb[:], logits_sb[:], Act.Tanh)

            m32_t = small_p.tile([BPC, N], i32)
            nc.sync.dma_start(m32_t[:], m32[:])
            m01b = small_p.tile([BPC, N], f32)
            nc.vector.tensor_copy(m01b[:], m32_t[:])
            mnegb = small_p.tile([BPC, N], f32)
            nc.vector.tensor_scalar(
                out=mnegb[:], in0=m01b[:], scalar1=-1.0, scalar2=-NEG,
                op0=Alu.add, op1=Alu.mult)

            # in place: logits = 10*tanh + mneg
            nc.vector.scalar_tensor_tensor(
                out=logits_sb[:], in0=logits_sb[:], scalar=10.0, in1=mnegb[:],
                op0=Alu.mult, op1=Alu.add)

            negmaxl = small_p.tile([BPC, 1], f32)
            nc.vector.tensor_reduce(out=negmaxl[:], in_=logits_sb[:],
                                    axis=Ax.X, op=Alu.max, negate=True)
            el = small_p.tile([BPC, N], f32)
            nc.scalar.activation(el[:], logits_sb[:], Act.Exp,
                                 bias=negmaxl[:, 0:1])
            nc.vector.tensor_tensor(out=el[:], in0=el[:], in1=m01b[:],
                                    op=Alu.mult)
            sl_sum = small_p.tile([BPC, 1], f32)
            nc.vector.tensor_reduce(out=sl_sum[:], in_=el[:], axis=Ax.X,
                                    op=Alu.add)
            probs_sb = small_p.tile([BPC, 1], f32)
            nc.vector.reciprocal(probs_sb[:], sl_sum[:])
            nc.sync.dma_start(probs_out[:], probs_sb[:])

            max8 = small_p.tile([BPC, 8], f32)
            nc.vector.max(max8[:], logits_sb[:])
            idx8 = small_p.tile([BPC, 8], u32)
            nc.vector.max_index(idx8[:], max8[:], logits_sb[:])
            vert_sb = small_p.tile([BPC, 1], i32)
            nc.vector.tensor_copy(vert_sb[:], idx8[:, 0:1].bitcast(i32))
            nc.sync.dma_start(vert_out[:], vert_sb[:])

    nc.finalize()
    return nc


def _get_program():
    if "nc" not in _PROG_CACHE:
        _PROG_CACHE["nc"] = _build_program()
    return _PROG_CACHE["nc"]


def _prep_core_inputs(inputs, core):
    """Pure layout transforms for one core's batch slice."""
    f32 = np.float32
    sl = slice(core * BPC, (core + 1) * BPC)
    h_g = np.asarray(inputs["h_g"], f32)[sl]
    first = np.asarray(inputs["first"], f32)[sl]
    last = np.asarray(inputs["last"], f32)[sl]
    context = np.asarray(inputs["context"], f32)[sl]
    K = np.asarray(inputs["K"], f32)[sl]
    V = np.asarray(inputs["V"], f32)[sl]
    K_lg = np.asarray(inputs["K_lg"], f32)[sl]
    mask = np.asarray(inputs["mask"], np.int32)[sl]

    h_c = np.concatenate([h_g, first, last, context], axis=1)      # [32, 386]
    hcT = np.zeros((KPAD, BPC), f32)
    hcT[: 3 * D + 2] = h_c.T

    Kn = np.ascontiguousarray(K.reshape(BPC * H, N * HD))
    Vt = np.ascontiguousarray(
        V.transpose(0, 1, 3, 2).reshape(BPC * H, HD * N))
    KlgT = np.ascontiguousarray(K_lg.transpose(0, 2, 1))           # [32,128,1024]
    mrep = np.ascontiguousarray(np.repeat(mask, H, axis=0))        # [256,1024]

    return {
        "hcT": hcT,
        "Kn": Kn,
        "Vt": Vt,
        "KlgT": KlgT,
        "mrep": mrep,
        "m32": np.ascontiguousarray(mask),
    }


def _shared_inputs(inputs):
    f32 = np.float32
    Wq = np.asarray(inputs["Wq"], f32)
    bq = np.asarray(inputs["bq"], f32)
    Wo = np.asarray(inputs["Wo"], f32)
    bo = np.asarray(inputs["bo"], f32)
    wqT = np.zeros((KPAD, D), f32)
    wqT[: 3 * D + 2] = Wq.T
    return {
        "wqT": wqT,
        "bq": np.ascontiguousarray(bq.reshape(D, 1)),
        "woT": np.ascontiguousarray(Wo.T),
        "bo": np.ascontiguousarray(bo.reshape(D, 1)),
        "ident": np.eye(128, dtype=f32),
    }


def make_in_maps(inputs):
    shared = _shared_inputs(inputs)
    return [dict(_prep_core_inputs(inputs, c), **shared) for c in range(NCORES)]


def _assemble(results):
    verts = np.concatenate([np.asarray(r["verts"], np.int32) for r in results])
    probs = np.concatenate([np.asarray(r["probs"], np.float32) for r in results])
    return verts.reshape(B, 1), probs.reshape(B, 1)


def run_spmd(inputs, trace=False, **kw):
    from concourse.bass_utils import run_bass_kernel_spmd

    nc = _get_program()
    in_maps = make_in_maps(inputs)
    br = run_bass_kernel_spmd(nc, in_maps, list(range(NCORES)), trace=trace, **kw)
    return br


def kernel(**inputs):
    br = run_spmd(inputs, trace=False)
    return _assemble(br.results)


# revision 51
# speedup vs baseline: 1.0797x; 1.0797x over previous
"""Trainium2 Bass kernel for nn_Decoder (single-query MHA + pointer head).

Contract: kernel(**inputs) takes the FULL unsharded numpy inputs (as produced
by the problem's setup_inputs) and returns the full output (vertexes, probs),
matching the reference up to fp32 rounding.

Strategy (pure data parallelism over batch, 8 NeuronCores, 32 batch each):
  - Host does LAYOUT ONLY: batch-slice, concat h_c, transposes of V/K_lg,
    zero padding, mask replication. All math runs on device in fp32.
  - Per core, partition layout (b_local, head) on 128 partitions x 2 blocks:
    scores (K dot q, grouped reduce) and the attention-weighted V sum run on
    Vector/GpSimd with n in the free dimension (row softmax is native).
  - Q / Wo projections run on the Tensor engine; pointer logits use M=1
    matmuls with the u2 column stationary and K_lg.T streaming as the moving
    operand, drained via scalar copies + small scatter DMAs.
  - argmax via DVE max/max_index (first-index tie-break == jnp.argmax).
"""

import numpy as np

B, N, D, H, HD = 256, 1024, 128, 8, 16
NCORES = 8
BPC = B // NCORES          # 32 batches per core
BLK_B = 16                 # batches per partition-block (16 b x 8 h = 128)
NBLK = BPC // BLK_B        # 2
KPAD = 512                 # 386 -> 512 (4 chunks of 128) for Q projection
NEG = -1.0e15
RSQ_D = float(1.0 / np.sqrt(128.0))

_PROG_CACHE = {}


def _build_program():
    """Build the (SPMD-identical) Bass program once."""
    import concourse.bass as bass
    import concourse.bacc as bacc
    import concourse.mybir as mybir
    from concourse.tile import TileContext

    f32 = mybir.dt.float32
    i32 = mybir.dt.int32
    u32 = mybir.dt.uint32
    Alu = mybir.AluOpType
    Act = mybir.ActivationFunctionType
    Ax = mybir.AxisListType

    # Bacc (not plain Bass): its compile() pass legalizes instruction-attached
    # semaphore waits (move_matmul_waits_to_ldweights, event semaphores) that
    # walrus codegen otherwise rejects ("Too many sync wait commands").
    nc = bacc.Bacc(None, target_bir_lowering=False)

    # ---- DRAM parameters (per-core) ----
    hcT = nc.declare_dram_parameter("hcT", [KPAD, BPC], f32, isOutput=False)
    wqT = nc.declare_dram_parameter("wqT", [KPAD, D], f32, isOutput=False)
    bq = nc.declare_dram_parameter("bq", [D, 1], f32, isOutput=False)
    woT = nc.declare_dram_parameter("woT", [D, D], f32, isOutput=False)
    bo = nc.declare_dram_parameter("bo", [D, 1], f32, isOutput=False)
    ident = nc.declare_dram_parameter("ident", [128, 128], f32, isOutput=False)
    Kn = nc.declare_dram_parameter("Kn", [BPC * H, N * HD], f32, isOutput=False)
    Vt = nc.declare_dram_parameter("Vt", [BPC * H, HD * N], f32, isOutput=False)
    KlgT = nc.declare_dram_parameter("KlgT", [BPC, D, N], f32, isOutput=False)
    mrep = nc.declare_dram_parameter("mrep", [BPC * H, N], i32, isOutput=False)
    m32 = nc.declare_dram_parameter("m32", [BPC, N], i32, isOutput=False)
    vert_out = nc.declare_dram_parameter("verts", [BPC, 1], i32, isOutput=True)
    probs_out = nc.declare_dram_parameter("probs", [BPC, 1], f32, isOutput=True)

    NC4 = 4096             # K/V free elems per chunk tile (256 n x 16 d)
    NCH = N // 256         # 4 chunks

    with TileContext(nc) as tc:
        import contextlib

        with contextlib.ExitStack() as ctx:
            const_p = ctx.enter_context(tc.tile_pool(name="const", bufs=1))
            small_p = ctx.enter_context(tc.tile_pool(name="small", bufs=1))
            blk_p = ctx.enter_context(tc.tile_pool(name="blk", bufs=2))
            kstream = ctx.enter_context(tc.tile_pool(name="kstream", bufs=2))
            vstream = ctx.enter_context(tc.tile_pool(name="vstream", bufs=2))
            prod_p = ctx.enter_context(tc.tile_pool(name="prod", bufs=2))
            klg_p = ctx.enter_context(tc.tile_pool(name="klg", bufs=3))
            psum_p = ctx.enter_context(
                tc.tile_pool(name="psum", bufs=1, space=bass.MemorySpace.PSUM)
            )
            psum_tr = ctx.enter_context(
                tc.tile_pool(name="psumtr", bufs=2, space=bass.MemorySpace.PSUM)
            )
            psum_lg = ctx.enter_context(
                tc.tile_pool(name="psumlg", bufs=4, space=bass.MemorySpace.PSUM)
            )
            dram_p = ctx.enter_context(
                tc.tile_pool(name="dram", bufs=1, space=bass.MemorySpace.DRAM)
            )

            # ---------- Phase Q: Q = 0.25*(h_c @ Wq.T + bq), per (b,h) ----------
            ident_t = const_p.tile([128, 128], f32)
            nc.sync.dma_start(ident_t[:], ident[:])

            bq_t = const_p.tile([D, 1], f32)
            nc.sync.dma_start(bq_t[:], bq[:])
            bq25 = const_p.tile([D, 1], f32)
            nc.scalar.mul(bq25[:], bq_t[:], 0.25)

            qt_ps = psum_p.tile([D, BPC], f32)  # Q.T accumulate over k-chunks
            for kc in range(KPAD // 128):
                wq_t = blk_p.tile([128, D], f32, name="wq_t")
                nc.sync.dma_start(wq_t[:], wqT[kc * 128:(kc + 1) * 128, :])
                hc_t = blk_p.tile([128, BPC], f32, name="hc_t")
                nc.sync.dma_start(hc_t[:], hcT[kc * 128:(kc + 1) * 128, :])
                nc.tensor.matmul(
                    qt_ps[:], wq_t[:], hc_t[:],
                    start=(kc == 0), stop=(kc == KPAD // 128 - 1),
                )
            qt_s = small_p.tile([D, BPC], f32)  # 0.25*(Q.T + bq), [(h d), b]
            nc.scalar.activation(qt_s[:], qt_ps[:], Act.Identity,
                                 bias=bq25[:, 0:1], scale=0.25)

            # transpose -> Q [b, (h d)] and roundtrip via DRAM to [(b h), d]
            q_tr_ps = psum_p.tile([BPC, D], f32)
            nc.tensor.transpose(q_tr_ps[:], qt_s[:], ident_t[:])
            q_sb = small_p.tile([BPC, D], f32)
            nc.scalar.copy(q_sb[:], q_tr_ps[:])
            q_dram = dram_p.tile([BPC, D], f32)
            nc.sync.dma_start(q_dram[:], q_sb[:])

            u_dram = dram_p.tile([BPC, D], f32)
            u2s = small_p.tile([D, BPC], f32)  # (Wo u + bo)/sqrt(D), [(d2), b]

            bo_t = const_p.tile([D, 1], f32)
            nc.sync.dma_start(bo_t[:], bo[:])
            bo_s = const_p.tile([D, 1], f32)
            nc.scalar.mul(bo_s[:], bo_t[:], RSQ_D)

            wo_t = const_p.tile([D, D], f32)
            nc.sync.dma_start(wo_t[:], woT[:])

            # pointer-logits staging: psum rows drained into [b, n]
            logits_sb = small_p.tile([BPC, N], f32)

            for blk in range(NBLK):
                rows = slice(blk * 128, (blk + 1) * 128)

                q_tile = blk_p.tile([128, HD], f32, name="q_tile")
                nc.sync.dma_start(
                    q_tile[:],
                    q_dram[blk * BLK_B:(blk + 1) * BLK_B, :]
                    .rearrange("b (h d) -> (b h) d", h=H),
                )

                # masks for this block
                mrep_t = blk_p.tile([128, N], i32, name="mrep_t")
                nc.sync.dma_start(mrep_t[:], mrep[rows, :])
                m01 = blk_p.tile([128, N], f32, name="m01")
                nc.vector.tensor_copy(m01[:], mrep_t[:])
                mneg = blk_p.tile([128, N], f32, name="mneg")
                nc.vector.tensor_scalar(
                    out=mneg[:], in0=m01[:], scalar1=-1.0, scalar2=-NEG,
                    op0=Alu.add, op1=Alu.mult,
                )

                # ---------- scores: s[(b h), n] = sum_d K * q ----------
                scores_raw = blk_p.tile([128, N], f32, name="scores_raw")
                for c in range(NCH):
                    ktile = kstream.tile([128, NC4], f32, name="ktile")
                    nc.sync.dma_start(
                        ktile[:], Kn[rows, c * NC4:(c + 1) * NC4])
                    kprod = prod_p.tile([128, NC4], f32, name="kprod",
                                        tag="prod")
                    kv = ktile[:].rearrange("p (n d) -> p n d", d=HD)
                    # multiply on GpSimd to offload the Vector engine
                    nc.gpsimd.tensor_tensor(
                        out=kprod[:].rearrange("p (n d) -> p n d", d=HD),
                        in0=kv,
                        in1=q_tile[:].unsqueeze(1).broadcast_to([128, 256, HD]),
                        op=Alu.mult,
                    )
                    nc.vector.tensor_reduce(
                        out=scores_raw[:, c * 256:(c + 1) * 256],
                        in_=kprod[:].rearrange("p (n d) -> p n d", d=HD),
                        axis=Ax.X, op=Alu.add,
                    )

                # masked scores (in place): scores_raw += mneg
                nc.vector.tensor_tensor(
                    out=scores_raw[:], in0=scores_raw[:], in1=mneg[:],
                    op=Alu.add)

                negmax = blk_p.tile([128, 1], f32, name="negmax")
                nc.vector.tensor_reduce(
                    out=negmax[:], in_=scores_raw[:], axis=Ax.X, op=Alu.max,
                    negate=True)

                e2 = blk_p.tile([128, N], f32, name="e2")
                nc.scalar.activation(e2[:], scores_raw[:], Act.Exp,
                                     bias=negmax[:, 0:1])
                # zero out masked lanes exactly (in place)
                nc.vector.tensor_tensor(out=e2[:], in0=e2[:], in1=m01[:],
                                        op=Alu.mult)
                s_sum = blk_p.tile([128, 1], f32, name="s_sum")
                nc.vector.tensor_reduce(out=s_sum[:], in_=e2[:], axis=Ax.X,
                                        op=Alu.add)
                rec_s = blk_p.tile([128, 1], f32, name="rec_s")
                nc.vector.reciprocal(rec_s[:], s_sum[:])

                # ---------- u[(b h), d] = (sum_n e2 * V) / S ----------
                part4 = blk_p.tile([128, HD, NCH], f32, name="part4")
                for c in range(NCH):
                    vtile = vstream.tile([128, NC4], f32, name="vtile")
                    nc.scalar.dma_start(
                        vtile[:].rearrange("p (d n) -> p d n", n=256),
                        Vt[rows, :].rearrange("p (d n) -> p d n", n=N)
                        [:, :, c * 256:(c + 1) * 256],
                    )
                    vprod = prod_p.tile([128, NC4], f32, name="vprod",
                                        tag="prod")
                    nc.vector.tensor_tensor(
                        out=vprod[:].rearrange("p (d n) -> p d n", n=256),
                        in0=vtile[:].rearrange("p (d n) -> p d n", n=256),
                        in1=e2[:, c * 256:(c + 1) * 256].unsqueeze(1)
                        .broadcast_to([128, HD, 256]),
                        op=Alu.mult,
                    )
                    nc.vector.tensor_reduce(
                        out=part4[:, :, c],
                        in_=vprod[:].rearrange("p (d n) -> p d n", n=256),
                        axis=Ax.X, op=Alu.add,
                    )
                usum = blk_p.tile([128, HD], f32, name="usum")
                nc.vector.tensor_reduce(out=usum[:], in_=part4[:], axis=Ax.X,
                                        op=Alu.add)
                u_blk = blk_p.tile([128, HD], f32, name="u_blk")
                nc.vector.tensor_tensor(
                    out=u_blk[:], in0=usum[:],
                    in1=rec_s[:, 0:1].broadcast_to([128, HD]), op=Alu.mult)

                nc.sync.dma_start(
                    u_dram[blk * BLK_B:(blk + 1) * BLK_B, :]
                    .rearrange("b (h d) -> (b h) d", h=H),
                    u_blk[:],
                )

                # ---------- u2 for this block: [(d2), b_blk] ----------
                u_plain = blk_p.tile([BLK_B, D], f32, name="u_plain")
                nc.sync.dma_start(
                    u_plain[:], u_dram[blk * BLK_B:(blk + 1) * BLK_B, :])
                uT_ps = psum_tr.tile([D, BLK_B], f32, name="uT_ps", bufs=1)
                nc.tensor.transpose(uT_ps[:], u_plain[:],
                                    ident_t[:BLK_B, :BLK_B])
                uT_sb = blk_p.tile([D, BLK_B], f32, name="uT_sb")
                nc.scalar.copy(uT_sb[:], uT_ps[:])
                u2_ps = psum_tr.tile([D, BLK_B], f32, name="u2_ps", bufs=1)
                nc.tensor.matmul(u2_ps[:], wo_t[:], uT_sb[:])
                nc.scalar.activation(
                    u2s[:, blk * BLK_B:(blk + 1) * BLK_B], u2_ps[:],
                    Act.Identity, bias=bo_s[:, 0:1], scale=RSQ_D)

                # ---------- pointer logits for this block's batches ----------
                # u2 column is the (tiny) stationary operand; K_lg.T streams
                # through the PE as the moving operand at full rate. The psum
                # row (partition 0) is staged to SBUF by the scalar engine,
                # then a small DMA scatters it into logits_sb[b].
                for bl in range(0, BLK_B, 2):
                    b = blk * BLK_B + bl
                    klg_t = klg_p.tile([D, 2 * N], f32, name="klg_t")
                    nc.sync.dma_start(
                        klg_t[:].rearrange("d (two n) -> d two n", two=2),
                        KlgT[b:b + 2, :, :].rearrange("two d n -> d two n"),
                    )
                    for j in range(2):
                        bb = b + j
                        stage = blk_p.tile([1, N], f32, name="lgrow", bufs=4)
                        for c in range(2):
                            lg_ps = psum_lg.tile([1, 512], f32, name="lg_ps")
                            nc.tensor.matmul(
                                lg_ps[:],
                                u2s[:, bb:bb + 1],
                                klg_t[:, j * N + c * 512:j * N + (c + 1) * 512],
                                start=True, stop=True,
                            )
                            nc.scalar.copy(
                                stage[0:1, c * 512:(c + 1) * 512], lg_ps[:])
                        nc.sync.dma_start(
                            logits_sb[bb:bb + 1, :], stage[0:1, :])

            # ---------- finish pointer head on [b, n] ----------
            nc.scalar.activation(logits_sb[:], logits_sb[:], Act.Tanh)

            m32_t = small_p.tile([BPC, N], i32)
            nc.sync.dma_start(m32_t[:], m32[:])
            m01b = small_p.tile([BPC, N], f32)
            nc.vector.tensor_copy(m01b[:], m32_t[:])
            mnegb = small_p.tile([BPC, N], f32)
            nc.vector.tensor_scalar(
                out=mnegb[:], in0=m01b[:], scalar1=-1.0, scalar2=-NEG,
                op0=Alu.add, op1=Alu.mult)

            # in place: logits = 10*tanh + mneg
            nc.vector.scalar_tensor_tensor(
                out=logits_sb[:], in0=logits_sb[:], scalar=10.0, in1=mnegb[:],
                op0=Alu.mult, op1=Alu.add)

            negmaxl = small_p.tile([BPC, 1], f32)
            nc.vector.tensor_reduce(out=negmaxl[:], in_=logits_sb[:],
                                    axis=Ax.X, op=Alu.max, negate=True)
            el = small_p.tile([BPC, N], f32)
            nc.scalar.activation(el[:], logits_sb[:], Act.Exp,
                                 bias=negmaxl[:, 0:1])
            nc.vector.tensor_tensor(out=el[:], in0=el[:], in1=m01b[:],
                                    op=Alu.mult)
            sl_sum = small_p.tile([BPC, 1], f32)
            nc.vector.tensor_reduce(out=sl_sum[:], in_=el[:], axis=Ax.X,
                                    op=Alu.add)
            probs_sb = small_p.tile([BPC, 1], f32)
            nc.vector.reciprocal(probs_sb[:], sl_sum[:])
            nc.sync.dma_start(probs_out[:], probs_sb[:])

            max8 = small_p.tile([BPC, 8], f32)
            nc.vector.max(max8[:], logits_sb[:])
            idx8 = small_p.tile([BPC, 8], u32)
            nc.vector.max_index(idx8[:], max8[:], logits_sb[:])
            vert_sb = small_p.tile([BPC, 1], i32)
            nc.vector.tensor_copy(vert_sb[:], idx8[:, 0:1].bitcast(i32))
            nc.sync.dma_start(vert_out[:], vert_sb[:])

    nc.finalize()
    return nc


def _get_program():
    if "nc" not in _PROG_CACHE:
        _PROG_CACHE["nc"] = _build_program()
    return _PROG_CACHE["nc"]


def _prep_core_inputs(inputs, core):
    """Pure layout transforms for one core's batch slice."""
    f32 = np.float32
    sl = slice(core * BPC, (core + 1) * BPC)
    h_g = np.asarray(inputs["h_g"], f32)[sl]
    first = np.asarray(inputs["first"], f32)[sl]
    last = np.asarray(inputs["last"], f32)[sl]
    context = np.asarray(inputs["context"], f32)[sl]
    K = np.asarray(inputs["K"], f32)[sl]
    V = np.asarray(inputs["V"], f32)[sl]
    K_lg = np.asarray(inputs["K_lg"], f32)[sl]
    mask = np.asarray(inputs["mask"], np.int32)[sl]

    h_c = np.concatenate([h_g, first, last, context], axis=1)      # [32, 386]
    hcT = np.zeros((KPAD, BPC), f32)
    hcT[: 3 * D + 2] = h_c.T

    Kn = np.ascontiguousarray(K.reshape(BPC * H, N * HD))
    Vt = np.ascontiguousarray(
        V.transpose(0, 1, 3, 2).reshape(BPC * H, HD * N))
    KlgT = np.ascontiguousarray(K_lg.transpose(0, 2, 1))           # [32,128,1024]
    mrep = np.ascontiguousarray(np.repeat(mask, H, axis=0))        # [256,1024]

    return {
        "hcT": hcT,
        "Kn": Kn,
        "Vt": Vt,
        "KlgT": KlgT,
        "mrep": mrep,
        "m32": np.ascontiguousarray(mask),
    }


def _shared_inputs(inputs):
    f32 = np.float32
    Wq = np.asarray(inputs["Wq"], f32)
    bq = np.asarray(inputs["bq"], f32)
    Wo = np.asarray(inputs["Wo"], f32)
    bo = np.asarray(inputs["bo"], f32)
    wqT = np.zeros((KPAD, D), f32)
    wqT[: 3 * D + 2] = Wq.T
    return {
        "wqT": wqT,
        "bq": np.ascontiguousarray(bq.reshape(D, 1)),
        "woT": np.ascontiguousarray(Wo.T),
        "bo": np.ascontiguousarray(bo.reshape(D, 1)),
        "ident": np.eye(128, dtype=f32),
    }


def make_in_maps(inputs):
    shared = _shared_inputs(inputs)
    return [dict(_prep_core_inputs(inputs, c), **shared) for c in range(NCORES)]


def _assemble(results):
    verts = np.concatenate([np.asarray(r["verts"], np.int32) for r in results])
    probs = np.concatenate([np.asarray(r["probs"], np.float32) for r in results])
    return verts.reshape(B, 1), probs.reshape(B, 1)


def run_spmd(inputs, trace=False, **kw):
    from concourse.bass_utils import run_bass_kernel_spmd

    nc = _get_program()
    in_maps = make_in_maps(inputs)
    br = run_bass_kernel_spmd(nc, in_maps, list(range(NCORES)), trace=trace, **kw)
    return br


def kernel(**inputs):
    br = run_spmd(inputs, trace=False)
    return _assemble(br.results)
